# revision 33
# baseline (speedup 1.0000x reference)
"""CommNet forward pass on 8 Trainium2 NeuronCores.

Data-parallel over the batch dim: 256 batch elems -> 32 per core
(= 2048 tokens of 64 agents each). All weights replicated per core.

Device layout is feature-major: activations live in SBUF as
[feature_chunk(128 partitions), tokens]. Host pre-transposes obs and all
weights so every DMA is contiguous, and folds:
  - the 1/N comm scaling into W_ih,
  - b_ih + b_hh for the r/z gates,
  - the (linear) value+decoder layers into one W_vd = W_dec @ W_val.

The kernel is software-pipelined over 4 token tiles of 512: in step s the
tensor engine runs gates for tile s-1, encoder+obs for tile s and the value
head for tile s-2, so it never waits on the vector/scalar chain of a single
tile. Key device tricks:
  - gate chunks accumulate into 4 rotating single-bank PSUM tiles, evicted
    immediately; the obs/enc PSUM share one double-buffered rotation;
  - the comm reduce/sub runs directly on the obs-layer PSUM (pre-bias), with
    the missing 63/64*b_obs term folded into the gate biases on the host;
  - the n-gate needs no second PSUM bank: tt = (gh+b_hn)*r is written in
    place onto the gh bank by the DVE and the gi matmuls accumulate on top,
    so tanh reads the finished pre-activation straight from PSUM;
  - the GRU blend's final add is absorbed into the value head
    (out = W_vd@n + W_vd@(z*(h-n))), and the z*(h-n) product is split
    between the gpsimd and vector engines;
  - dummy matmuls during the initial weight-DMA window pre-warm the PE HAM
    clock gate, and dummy activations hoist the act-table loads into the
    same dead time.
"""

import numpy as np
import ml_dtypes

import concourse.bass as bass
import concourse.bacc as bacc
import concourse.mybir as mybir
import concourse.tile as tile
from concourse.bass import ts
from concourse.bass_utils import run_bass_kernel_spmd

N_CORES = 8
B, NA, D_IN = 256, 64, 128     # batch, agents, input dim
H0 = 256                       # hidden dim
H2 = 64                        # output dim
T_C = B * NA // N_CORES        # tokens per core (2048)
TT = 512                       # token tile (= max fp32 PSUM bank width)
NT = T_C // TT                 # token tiles per core (4)
NB = TT // NA                  # batch elems per token tile (8)
N_WARM = 5                     # HAM clock-gate warmup matmuls

F32 = mybir.dt.float32
BF16 = mybir.dt.bfloat16
AF = mybir.ActivationFunctionType
ADD = mybir.AluOpType.add
MULT = mybir.AluOpType.mult

# Set by test harness to collect a profile; kernel() stores timing here.
TRACE = False
LAST_EXEC_NS = None
LAST_RESULTS = None

_PROGRAM_CACHE = {}


def _build_program():
    nc = bacc.Bacc("TRN2", target_bir_lowering=False)

    obs_d = nc.dram_tensor("obs_t", [128, T_C], BF16, kind="ExternalInput")
    wenc_d = nc.dram_tensor("wenc", [128, 2, 128], BF16, kind="ExternalInput")
    wobs_d = nc.dram_tensor("wobs", [128, 2, 256], BF16, kind="ExternalInput")
    wihrz_d = nc.dram_tensor("wihrz", [128, 2, 512], BF16, kind="ExternalInput")
    whhrz_d = nc.dram_tensor("whhrz", [128, 2, 512], BF16, kind="ExternalInput")
    wihn_d = nc.dram_tensor("wihn", [128, 2, 256], BF16, kind="ExternalInput")
    whhn_d = nc.dram_tensor("whhn", [128, 2, 256], BF16, kind="ExternalInput")
    wvd_d = nc.dram_tensor("wvd", [128, 2, 64], BF16, kind="ExternalInput")
    bias_d = nc.dram_tensor("bias", [128, 12], F32, kind="ExternalInput")
    bvd_d = nc.dram_tensor("bvd", [64, 1], F32, kind="ExternalInput")
    out_d = nc.dram_tensor("out_t", [64, T_C], F32, kind="ExternalOutput")

    with tile.TileContext(nc) as tc:
        with (
            tc.tile_pool(name="wpool", bufs=1) as wp,
            tc.tile_pool(name="io", bufs=1) as iop,
            tc.tile_pool(name="act", bufs=1) as ap,
            tc.tile_pool(name="psG", bufs=4, space="PSUM") as psG,
            tc.tile_pool(name="psH", bufs=2, space="PSUM") as psH,
        ):
            # ---- DMA prologue: split weights over 3 queues ----
            def wload(dram, shape, tag, dt=BF16, eng=nc.gpsimd):
                t = wp.tile(shape, dt, tag=tag)
                eng.dma_start(out=t, in_=dram[:])
                return t

            # The scalar queue is kept DMA-free: its act-table loads would
            # otherwise delay weight issue.
            # obs tile 0 is the fill critical path: land it as two parallel
            # half-DMAs on different queues so the encoder can start on the
            # first half ~0.7us sooner.
            obs_sb = []
            o0 = iop.tile([128, TT], BF16, tag="obs0")
            nc.sync.dma_start(out=o0[:, 0:TT // 2], in_=obs_d[:, 0:TT // 2])
            nc.scalar.dma_start(out=o0[:, TT // 2:TT],
                                in_=obs_d[:, TT // 2:TT])
            obs_sb.append(o0)
            wenc = wload(wenc_d, [128, 2, 128], "wenc", eng=nc.gpsimd)
            whhrz = wload(whhrz_d, [128, 2, 512], "whhrz", eng=nc.sync)
            bias = wload(bias_d, [128, 12], "bias", F32, eng=nc.sync)
            wihrz = wload(wihrz_d, [128, 2, 512], "wihrz", eng=nc.gpsimd)
            wobs = wload(wobs_d, [128, 2, 256], "wobs", eng=nc.sync)
            whhn = wload(whhn_d, [128, 2, 256], "whhn", eng=nc.sync)
            wihn = wload(wihn_d, [128, 2, 256], "wihn", eng=nc.gpsimd)
            wvd = wload(wvd_d, [128, 2, 64], "wvd", eng=nc.gpsimd)
            bvd = wload(bvd_d, [64, 1], "bvd", F32, eng=nc.gpsimd)
            for i in range(1, NT):
                o = iop.tile([128, TT], BF16, tag=f"obs{i}")
                nc.sync.dma_start(out=o, in_=obs_d[:, ts(i, TT)])
                obs_sb.append(o)

            benc, bobs, brz = bias[:, 0:2], bias[:, 2:4], bias[:, 4:8]
            bin_, bhn = bias[:, 8:10], bias[:, 10:12]

            # ---- act-table hoist + PE clock warmup (runs in DMA dead time)
            warm = ap.tile([128, TT], BF16, tag="warm")
            nc.vector.memset(warm, 0.0)
            dump = ap.tile([128, 4], F32, tag="dump")
            nc.scalar.activation(dump[:, 0:1], warm[:, 0:1], AF.Sigmoid)
            nc.scalar.activation(dump[:, 1:2], warm[:, 0:1], AF.Tanh)
            nc.scalar.activation(dump[:, 2:3], warm[:, 0:1], AF.Relu)
            nc.scalar.activation(dump[:, 3:4], warm[:, 0:1], AF.Identity)
            ps_warm = psG.tile([128, TT], F32, tag="g")
            for _ in range(N_WARM):
                nc.tensor.matmul(ps_warm, warm[:, 0:128], warm,
                                 start=True, stop=True)

            # ---- per-tile state ----
            xts = [None] * NT
            hts = [None] * NT
            cts = [None] * NT
            Sts = [None] * NT
            rts = [None] * NT
            zts = [None] * NT
            tts = [None] * NT
            t2s = [None] * NT
            nts = [None] * NT
            h2s = [None] * NT
            ps_ghs = [None] * NT
            ps_gis = [None] * NT
            ps_rz = [None] * NT

            def warm_fill(n):
                for _ in range(n):
                    nc.tensor.matmul(ps_warm, warm[:, 0:128], warm,
                                     start=True, stop=True)

            def emit_enc(t):
                psx = psH.tile([128, 2, TT], F32, tag="h", name="psx")
                if t == 0:
                    # per-half matmuls, m-major: relu(m0) can start two
                    # matmuls earlier
                    half = TT // 2
                    for m in range(2):
                        for hh in range(2):
                            sl = slice(hh * half, (hh + 1) * half)
                            nc.tensor.matmul(psx[:, m, sl], wenc[:, m, :],
                                             obs_sb[t][:, sl],
                                             start=True, stop=True)
                else:
                    for m in range(2):
                        nc.tensor.matmul(psx[:, m, :], wenc[:, m, :],
                                         obs_sb[t], start=True, stop=True)
                xt = ap.tile([128, 2, TT], BF16, tag="x", bufs=2, name="xt")
                xts[t] = (psx, xt)

            def emit_step(s):
                g = s - 1       # gate tile
                cur = s         # obs/comm tile (enc ran at end of step s-1)
                v = s - 2       # value-head tile
                nxt = s + 1     # encoder tile emitted at the tail

                # -- scalar head: relu for cur (enc PSUM from prev step).
                # high_priority: the greedy scheduler otherwise parks relu
                # behind the gate sigmoids and the obs matmuls stall on it.
                if 0 <= cur < NT:
                    psx, xt = xts[cur]
                    with tc.high_priority():
                        for m in range(2):
                            nc.scalar.activation(xt[:, m, :], psx[:, m, :],
                                                 AF.Relu,
                                                 bias=benc[:, m:m + 1])

                # -- tensor: r gates (h-parts then c-parts, chunk-serial)
                if 0 <= g < NT:
                    ht, ct = hts[g], cts[g]
                    prz = [None] * 2
                    for j in range(2):          # r0, r1
                        p = psG.tile([128, TT], F32, tag="g")
                        prz[j] = p
                        for k in range(2):
                            nc.tensor.matmul(p, whhrz[:, k, ts(j, 128)],
                                             ht[:, k, :], start=(k == 0),
                                             stop=False)
                        for k in range(2):
                            nc.tensor.matmul(p, wihrz[:, k, ts(j, 128)],
                                             ct[:, k, :, :], start=False,
                                             stop=(k == 1))
                    # scalar: sigmoid r
                    rt = ap.tile([128, 2, TT], BF16, tag="rt")
                    rts[g] = rt
                    for j in range(2):
                        nc.scalar.activation(rt[:, j, :], prz[j],
                                             AF.Sigmoid, bias=brz[:, j:j + 1])

                # -- tensor: obs matmuls for cur; vector: comm off PSUM
                if 0 <= cur < NT:
                    psx, xt = xts[cur]
                    psh = psH.tile([128, 2, TT], F32, tag="h", name="psh")
                    ht_c = ap.tile([128, 2, TT], BF16, tag="h2sb", bufs=2,
                                   name="ht_c")
                    hts[cur] = ht_c
                    for m in range(2):
                        for k in range(2):
                            nc.tensor.matmul(psh[:, m, :],
                                             wobs[:, k, ts(m, 128)],
                                             xt[:, k, :], start=(k == 0),
                                             stop=(k == 1))
                    if s == 0:
                        warm_fill(4)    # keep the PE busy through comm fill
                    # vector: comm reduce + broadcast-sub straight off PSUM
                    # (c' = sum_a h0 - h0; the 63/64*b_obs term is folded
                    #  into the gate biases on the host), then evict ht.
                    ph4 = psh[:, :, :].rearrange("p m (b n) -> p m b n", n=NA)
                    St = ap.tile([128, 2, NB], F32, tag="S", bufs=2)
                    ct_c = ap.tile([128, 2, NB, NA], BF16, tag="c", bufs=2,
                                   name="ct_c")
                    Sts[cur], cts[cur] = St, ct_c
                    if cur <= 1:
                        # ramp tiles: ht evict on the (still idle) scalar
                        # engine so the r/z h-part matmuls start while the
                        # DVE does the comm; per-chunk singles let each ct
                        # chunk release its ih matmul via subtile deps.
                        with tc.high_priority():
                            for m in range(2):
                                nc.scalar.activation(ht_c[:, m, :],
                                                     psh[:, m, :],
                                                     AF.Identity,
                                                     bias=bobs[:, m:m + 1])
                            for k in range(2):
                                nc.vector.reduce_sum(
                                    out=St[:, k, :], in_=ph4[:, k, :, :],
                                    axis=mybir.AxisListType.X)
                                nc.vector.tensor_sub(
                                    ct_c[:, k, :, :],
                                    St[:, k, :, None].broadcast_to(
                                        [128, NB, NA]),
                                    ph4[:, k, :, :])
                    else:
                        # steady tiles: one instruction per op class; the
                        # DVE pays ~150 fixed cycles per op, pairs save
                        # ~0.4us/tile
                        with tc.high_priority():
                            nc.vector.reduce_sum(out=St, in_=ph4,
                                                 axis=mybir.AxisListType.X)
                            nc.vector.tensor_sub(
                                ct_c,
                                St[:, :, :, None].broadcast_to(
                                    [128, 2, NB, NA]),
                                ph4)
                        nc.vector.tensor_add(
                            ht_c, psh,
                            bobs[:, 0:2, None].broadcast_to([128, 2, TT]))

                # -- tensor: encoder for the next tile (psx shares the psH
                # rotation, freed by the early-step relu, so this can be
                # scheduled mid-step instead of at the tail)
                if 0 <= nxt < NT:
                    emit_enc(nxt)
                if s == 0:
                    # extra clock-warming filler for the tile-0 fill gaps
                    warm_fill(3)

                # -- tensor: z gates BEFORE gh: the psG rotation then hands
                # the next step's r banks a sigmoid-freed bank instead of a
                # tanh-freed one (tanh lands latest in the step)
                if 0 <= g < NT:
                    ht, ct = hts[g], cts[g]
                    zt = ap.tile([128, 2, TT], BF16, tag="zt")
                    zts[g] = zt
                    for j in range(2):
                        p = psG.tile([128, TT], F32, tag="g")
                        for k in range(2):
                            nc.tensor.matmul(p, whhrz[:, k, ts(2 + j, 128)],
                                             ht[:, k, :], start=(k == 0),
                                             stop=False)
                        for k in range(2):
                            nc.tensor.matmul(p, wihrz[:, k, ts(2 + j, 128)],
                                             ct[:, k, :, :], start=False,
                                             stop=(k == 1))
                        nc.scalar.activation(zt[:, j, :], p, AF.Sigmoid,
                                             bias=brz[:, 2 + j:3 + j])

                    # tensor: gh (n-gate h-part); vector writes tt in place
                    pgh = [None] * 2
                    for m in range(2):
                        p = psG.tile([128, TT], F32, tag="g")
                        pgh[m] = p
                        for k in range(2):
                            nc.tensor.matmul(p, whhn[:, k, ts(m, 128)],
                                             ht[:, k, :], start=(k == 0),
                                             stop=(k == 1))
                    ps_ghs[g] = pgh
                    # vector: tt = (gh + b_hn) * r written IN PLACE onto the
                    # gh PSUM bank; the gi matmuls then accumulate on top and
                    # tanh reads the final value with the b_in bias.
                    for m in range(2):
                        nc.vector.scalar_tensor_tensor(
                            out=pgh[m], in0=pgh[m],
                            scalar=bhn[:, m:m + 1], in1=rts[g][:, m, :],
                            op0=ADD, op1=MULT)

                    # tensor: gi accumulates onto tt in the gh banks
                    nt_ = ap.tile([128, 2, TT], BF16, tag="nt", bufs=2)
                    nts[g] = nt_
                    for m in range(2):
                        for k in range(2):
                            nc.tensor.matmul(pgh[m], wihn[:, k, ts(m, 128)],
                                             ct[:, k, :, :], start=False,
                                             stop=(k == 1))
                        # scalar: n = tanh(tt + gi + b_in)
                        nc.scalar.activation(nt_[:, m, :], pgh[m], AF.Tanh,
                                             bias=bin_[:, m:m + 1])

                    # GRU blend: d = h - n, e = z*d. The final h' = n + e is
                    # absorbed into the value head (out = Wvd@n + Wvd@e) so
                    # no h2 materialization is needed. Last tile's m=1 chunk
                    # runs on vector to halve drain latency.
                    dt_ = ap.tile([128, 2, TT], BF16, tag="dt")
                    et = ap.tile([128, 2, TT], BF16, tag="et", bufs=2)
                    h2s[g] = et
                    for m in range(2):
                        eng = (nc.vector if (g == NT - 1
                                             or (g == NT - 2 and m == 1))
                               else nc.gpsimd)
                        eng.tensor_sub(dt_[:, m, :], hts[g][:, m, :],
                                       nts[g][:, m, :])
                        eng.tensor_mul(et[:, m, :], zts[g][:, m, :],
                                       dt_[:, m, :])

                # -- tensor: value head for tile v; scalar evict + DMA out
                if 0 <= v < NT:
                    # value PSUM from the psH pool: drops psG to 6 allocs
                    # per step so the z/gh matmuls stop waiting on sigmoid
                    # bank releases
                    ps_o2 = psH.tile([128, 2, TT], F32, tag="h", name="ps_o")
                    ps_o = ps_o2[:, 0, :]
                    for k in range(2):
                        nc.tensor.matmul(ps_o[:64, :], wvd[:, k, :],
                                         nts[v][:, k, :], start=(k == 0),
                                         stop=False)
                    for k in range(2):
                        nc.tensor.matmul(ps_o[:64, :], wvd[:, k, :],
                                         h2s[v][:, k, :], start=False,
                                         stop=(k == 1))
                    osb = iop.tile([64, TT], F32, tag="osb", bufs=2)
                    if v == NT - 1:
                        # last tile: evict + store in halves so the final
                        # DMA overlaps the eviction instead of serializing
                        half = TT // 2
                        for hh in range(2):
                            sl = slice(hh * half, (hh + 1) * half)
                            if hh == 0:
                                nc.scalar.activation(osb[:, sl],
                                                     ps_o[:64, sl],
                                                     AF.Identity,
                                                     bias=bvd[:, 0:1])
                                eng = nc.sync
                            else:
                                nc.vector.tensor_scalar_add(osb[:, sl],
                                                            ps_o[:64, sl],
                                                            bvd[:, 0:1])
                                eng = nc.scalar
                            eng.dma_start(
                                out=out_d[:, v * TT + hh * half:
                                          v * TT + (hh + 1) * half],
                                in_=osb[:, sl])
                    elif v == NT - 2:
                        # this evict would sit between stt(3) and the last
                        # blend in the DVE queue and delay the drain; the
                        # scalar engine is free here
                        nc.scalar.activation(osb, ps_o[:64, :], AF.Identity,
                                             bias=bvd[:, 0:1])
                        nc.sync.dma_start(out=out_d[:, ts(v, TT)], in_=osb)
                    else:
                        nc.vector.tensor_scalar_add(osb, ps_o[:64, :],
                                                    bvd[:, 0:1])
                        nc.sync.dma_start(out=out_d[:, ts(v, TT)], in_=osb)

            # encoder for tile 0 + a couple of fill matmuls to cover the
            # relu roundtrip before the step loop starts obs(0)
            emit_enc(0)
            warm_fill(2)
            for s in range(NT + 2):
                emit_step(s)

    nc.finalize()
    return nc


def _prep_maps(obs, W_enc, b_enc, W_obs, b_obs, W_ih, b_ih, W_hh, b_hh,
               W_val, b_val, W_dec, b_dec):
    f = np.float32
    obs = np.asarray(obs, f)
    tok = np.ascontiguousarray(obs.reshape(B * NA, D_IN))

    def rr(a):
        # Matmul operands are bf16 on device; round once on host (RNE).
        return np.ascontiguousarray(np.asarray(a, f).astype(ml_dtypes.bfloat16))

    def kc(w):  # [K, M] -> [128, K/128, M] partition-major k-chunks
        k, m = w.shape
        return np.ascontiguousarray(w.reshape(k // 128, 128, m).transpose(1, 0, 2))

    W_ih = np.asarray(W_ih, np.float64)
    W_hh = np.asarray(W_hh, np.float64)
    scale = 1.0 / NA

    wenc = rr(np.ascontiguousarray(np.asarray(W_enc, f).T).reshape(128, 2, 128))
    wobs = rr(kc(np.asarray(W_obs, f).T.astype(f)))
    wihrz = rr(kc((W_ih[:512].T * scale).astype(f)))
    whhrz = rr(kc(W_hh[:512].T.astype(f)))
    wihn = rr(kc((W_ih[512:].T * scale).astype(f)))
    whhn = rr(kc(W_hh[512:].T.astype(f)))
    W_vd = (np.asarray(W_dec, np.float64) @ np.asarray(W_val, np.float64))
    wvd = rr(kc(W_vd.T.astype(f)))
    b_vd = (np.asarray(W_dec, np.float64) @ np.asarray(b_val, np.float64)
            + np.asarray(b_dec, np.float64)).astype(f)

    def bc(b, n):  # [n*128] -> [128, n]
        return np.ascontiguousarray(np.asarray(b, f).reshape(n, 128).T)

    # The on-device comm term is c' = sum_a h0 - h0 computed on the obs-layer
    # PSUM (pre-bias h0); the missing 63/64 * b_obs contribution is folded
    # through W_ih into the gate biases here.
    cobs = np.asarray(b_obs, np.float64) * (63.0 / 64.0)
    gi_fold = W_ih @ cobs          # [768]
    brz_f = (np.asarray(b_ih, np.float64)
             + np.asarray(b_hh, np.float64))[:512] + gi_fold[:512]
    bin_f = np.asarray(b_ih, np.float64)[512:] + gi_fold[512:]
    bias_pack = np.concatenate([
        bc(b_enc, 2), bc(b_obs, 2),
        bc(brz_f.astype(f), 4),
        bc(bin_f.astype(f), 2),
        bc(np.asarray(b_hh, f)[512:], 2),
    ], axis=1)  # [128, 12]
    shared = {
        "wenc": wenc, "wobs": wobs, "wihrz": wihrz, "whhrz": whhrz,
        "wihn": wihn, "whhn": whhn, "wvd": wvd,
        "bias": np.ascontiguousarray(bias_pack),
        "bvd": np.ascontiguousarray(b_vd.reshape(64, 1)),
    }
    in_maps = []
    for ci in range(N_CORES):
        sh = rr(np.ascontiguousarray(tok[ci * T_C:(ci + 1) * T_C].T))  # [128, T_C]
        in_maps.append({**shared, "obs_t": sh})
    return in_maps


def kernel(**inputs):
    global LAST_EXEC_NS, LAST_RESULTS
    if "nc" not in _PROGRAM_CACHE:
        _PROGRAM_CACHE["nc"] = _build_program()
    nc = _PROGRAM_CACHE["nc"]

    in_maps = _prep_maps(**inputs)
    res = run_bass_kernel_spmd(nc, in_maps, list(range(N_CORES)), trace=TRACE)
    LAST_EXEC_NS = res.exec_time_ns
    LAST_RESULTS = res

    parts = []
    for ci in range(N_CORES):
        o = res.results[ci]["out_t"]            # [64, T_C]
        parts.append(np.ascontiguousarray(o.T).reshape(B // N_CORES, NA, H2))
    return np.concatenate(parts, axis=0).astype(np.float32)



# revision 34
# speedup vs baseline: 1.0051x; 1.0051x over previous
"""CommNet forward pass on 8 Trainium2 NeuronCores.

Data-parallel over the batch dim: 256 batch elems -> 32 per core
(= 2048 tokens of 64 agents each). All weights replicated per core.

Device layout is feature-major: activations live in SBUF as
[feature_chunk(128 partitions), tokens]. Host pre-transposes obs and all
weights so every DMA is contiguous, and folds:
  - the 1/N comm scaling into W_ih,
  - b_ih + b_hh for the r/z gates,
  - the (linear) value+decoder layers into one W_vd = W_dec @ W_val.

The kernel is software-pipelined over 4 token tiles of 512: in step s the
tensor engine runs gates for tile s-1, encoder+obs for tile s and the value
head for tile s-2, so it never waits on the vector/scalar chain of a single
tile. Key device tricks:
  - gate chunks accumulate into 4 rotating single-bank PSUM tiles, evicted
    immediately; the obs/enc PSUM share one double-buffered rotation;
  - the comm reduce/sub runs directly on the obs-layer PSUM (pre-bias), with
    the missing 63/64*b_obs term folded into the gate biases on the host;
  - the n-gate needs no second PSUM bank: tt = (gh+b_hn)*r is written in
    place onto the gh bank by the DVE and the gi matmuls accumulate on top,
    so tanh reads the finished pre-activation straight from PSUM;
  - the GRU blend's final add is absorbed into the value head
    (out = W_vd@n + W_vd@(z*(h-n))), and the z*(h-n) product is split
    between the gpsimd and vector engines;
  - dummy matmuls during the initial weight-DMA window pre-warm the PE HAM
    clock gate, and dummy activations hoist the act-table loads into the
    same dead time.
"""

import numpy as np
import ml_dtypes

import concourse.bass as bass
import concourse.bacc as bacc
import concourse.mybir as mybir
import concourse.tile as tile
from concourse.bass import ts
from concourse.bass_utils import run_bass_kernel_spmd

N_CORES = 8
B, NA, D_IN = 256, 64, 128     # batch, agents, input dim
H0 = 256                       # hidden dim
H2 = 64                        # output dim
T_C = B * NA // N_CORES        # tokens per core (2048)
TT = 512                       # token tile (= max fp32 PSUM bank width)
NT = T_C // TT                 # token tiles per core (4)
NB = TT // NA                  # batch elems per token tile (8)
N_WARM = 5                     # HAM clock-gate warmup matmuls

F32 = mybir.dt.float32
BF16 = mybir.dt.bfloat16
AF = mybir.ActivationFunctionType
ADD = mybir.AluOpType.add
MULT = mybir.AluOpType.mult

# Set by test harness to collect a profile; kernel() stores timing here.
TRACE = False
LAST_EXEC_NS = None
LAST_RESULTS = None

_PROGRAM_CACHE = {}


def _build_program():
    nc = bacc.Bacc("TRN2", target_bir_lowering=False)

    obs_d = nc.dram_tensor("obs_t", [128, T_C], BF16, kind="ExternalInput")
    wenc_d = nc.dram_tensor("wenc", [128, 2, 128], BF16, kind="ExternalInput")
    wobs_d = nc.dram_tensor("wobs", [128, 2, 256], BF16, kind="ExternalInput")
    wihrz_d = nc.dram_tensor("wihrz", [128, 2, 512], BF16, kind="ExternalInput")
    whhrz_d = nc.dram_tensor("whhrz", [128, 2, 512], BF16, kind="ExternalInput")
    wihn_d = nc.dram_tensor("wihn", [128, 2, 256], BF16, kind="ExternalInput")
    whhn_d = nc.dram_tensor("whhn", [128, 2, 256], BF16, kind="ExternalInput")
    wvd_d = nc.dram_tensor("wvd", [128, 2, 64], BF16, kind="ExternalInput")
    bias_d = nc.dram_tensor("bias", [128, 12], F32, kind="ExternalInput")
    bvd_d = nc.dram_tensor("bvd", [64, 1], F32, kind="ExternalInput")
    out_d = nc.dram_tensor("out_t", [64, T_C], F32, kind="ExternalOutput")

    with tile.TileContext(nc) as tc:
        with (
            tc.tile_pool(name="wpool", bufs=1) as wp,
            tc.tile_pool(name="io", bufs=1) as iop,
            tc.tile_pool(name="act", bufs=1) as ap,
            tc.tile_pool(name="psG", bufs=4, space="PSUM") as psG,
            tc.tile_pool(name="psH", bufs=2, space="PSUM") as psH,
        ):
            # ---- DMA prologue: split weights over 3 queues ----
            def wload(dram, shape, tag, dt=BF16, eng=nc.gpsimd):
                t = wp.tile(shape, dt, tag=tag)
                eng.dma_start(out=t, in_=dram[:])
                return t

            # The scalar queue is kept DMA-free: its act-table loads would
            # otherwise delay weight issue.
            # obs tile 0 is the fill critical path: land it as two parallel
            # half-DMAs on different queues so the encoder can start on the
            # first half ~0.7us sooner.
            obs_sb = []
            o0 = iop.tile([128, TT], BF16, tag="obs0")
            nc.sync.dma_start(out=o0[:, 0:TT // 2], in_=obs_d[:, 0:TT // 2])
            nc.scalar.dma_start(out=o0[:, TT // 2:TT],
                                in_=obs_d[:, TT // 2:TT])
            obs_sb.append(o0)
            wenc = wload(wenc_d, [128, 2, 128], "wenc", eng=nc.gpsimd)
            whhrz = wload(whhrz_d, [128, 2, 512], "whhrz", eng=nc.sync)
            bias = wload(bias_d, [128, 12], "bias", F32, eng=nc.sync)
            wihrz = wload(wihrz_d, [128, 2, 512], "wihrz", eng=nc.gpsimd)
            wobs = wload(wobs_d, [128, 2, 256], "wobs", eng=nc.sync)
            whhn = wload(whhn_d, [128, 2, 256], "whhn", eng=nc.sync)
            wihn = wload(wihn_d, [128, 2, 256], "wihn", eng=nc.gpsimd)
            wvd = wload(wvd_d, [128, 2, 64], "wvd", eng=nc.gpsimd)
            bvd = wload(bvd_d, [64, 1], "bvd", F32, eng=nc.gpsimd)
            for i in range(1, NT):
                o = iop.tile([128, TT], BF16, tag=f"obs{i}")
                nc.sync.dma_start(out=o, in_=obs_d[:, ts(i, TT)])
                obs_sb.append(o)

            benc, bobs, brz = bias[:, 0:2], bias[:, 2:4], bias[:, 4:8]
            bin_, bhn = bias[:, 8:10], bias[:, 10:12]

            # ---- act-table hoist + PE clock warmup (runs in DMA dead time)
            warm = ap.tile([128, TT], BF16, tag="warm")
            nc.vector.memset(warm, 0.0)
            dump = ap.tile([128, 4], F32, tag="dump")
            nc.scalar.activation(dump[:, 0:1], warm[:, 0:1], AF.Sigmoid)
            nc.scalar.activation(dump[:, 1:2], warm[:, 0:1], AF.Tanh)
            nc.scalar.activation(dump[:, 2:3], warm[:, 0:1], AF.Relu)
            nc.scalar.activation(dump[:, 3:4], warm[:, 0:1], AF.Identity)
            ps_warm = psG.tile([128, TT], F32, tag="g")
            for _ in range(N_WARM):
                nc.tensor.matmul(ps_warm, warm[:, 0:128], warm,
                                 start=True, stop=True)

            # ---- per-tile state ----
            xts = [None] * NT
            hts = [None] * NT
            cts = [None] * NT
            Sts = [None] * NT
            rts = [None] * NT
            zts = [None] * NT
            tts = [None] * NT
            t2s = [None] * NT
            nts = [None] * NT
            h2s = [None] * NT
            ps_ghs = [None] * NT
            ps_gis = [None] * NT
            ps_rz = [None] * NT

            def warm_fill(n):
                for _ in range(n):
                    nc.tensor.matmul(ps_warm, warm[:, 0:128], warm,
                                     start=True, stop=True)

            def emit_enc(t):
                psx = psH.tile([128, 2, TT], F32, tag="h", name="psx")
                if t == 0:
                    # per-half matmuls, m-major: relu(m0) can start two
                    # matmuls earlier
                    half = TT // 2
                    for m in range(2):
                        for hh in range(2):
                            sl = slice(hh * half, (hh + 1) * half)
                            nc.tensor.matmul(psx[:, m, sl], wenc[:, m, :],
                                             obs_sb[t][:, sl],
                                             start=True, stop=True)
                else:
                    for m in range(2):
                        nc.tensor.matmul(psx[:, m, :], wenc[:, m, :],
                                         obs_sb[t], start=True, stop=True)
                xt = ap.tile([128, 2, TT], BF16, tag="x", bufs=2, name="xt")
                xts[t] = (psx, xt)

            def emit_step(s):
                g = s - 1       # gate tile
                cur = s         # obs/comm tile (enc ran at end of step s-1)
                v = s - 2       # value-head tile
                nxt = s + 1     # encoder tile emitted at the tail

                # -- scalar head: relu for cur (enc PSUM from prev step).
                # high_priority: the greedy scheduler otherwise parks relu
                # behind the gate sigmoids and the obs matmuls stall on it.
                if 0 <= cur < NT:
                    psx, xt = xts[cur]
                    with tc.high_priority():
                        for m in range(2):
                            nc.scalar.activation(xt[:, m, :], psx[:, m, :],
                                                 AF.Relu,
                                                 bias=benc[:, m:m + 1])

                # -- tensor: r gates (h-parts then c-parts, chunk-serial)
                if 0 <= g < NT:
                    ht, ct = hts[g], cts[g]
                    prz = [None] * 2
                    for j in range(2):          # r0, r1
                        p = psG.tile([128, TT], F32, tag="g")
                        prz[j] = p
                        for k in range(2):
                            nc.tensor.matmul(p, whhrz[:, k, ts(j, 128)],
                                             ht[:, k, :], start=(k == 0),
                                             stop=False)
                        for k in range(2):
                            nc.tensor.matmul(p, wihrz[:, k, ts(j, 128)],
                                             ct[:, k, :, :], start=False,
                                             stop=(k == 1))
                    # scalar: sigmoid r
                    rt = ap.tile([128, 2, TT], BF16, tag="rt")
                    rts[g] = rt
                    for j in range(2):
                        nc.scalar.activation(rt[:, j, :], prz[j],
                                             AF.Sigmoid, bias=brz[:, j:j + 1])

                # -- tensor: obs matmuls for cur; vector: comm off PSUM
                if 0 <= cur < NT:
                    psx, xt = xts[cur]
                    psh = psH.tile([128, 2, TT], F32, tag="h", name="psh")
                    ht_c = ap.tile([128, 2, TT], BF16, tag="h2sb", bufs=2,
                                   name="ht_c")
                    hts[cur] = ht_c
                    for m in range(2):
                        for k in range(2):
                            nc.tensor.matmul(psh[:, m, :],
                                             wobs[:, k, ts(m, 128)],
                                             xt[:, k, :], start=(k == 0),
                                             stop=(k == 1))
                    if s == 0:
                        warm_fill(4)    # keep the PE busy through comm fill
                    # vector: comm reduce + broadcast-sub straight off PSUM
                    # (c' = sum_a h0 - h0; the 63/64*b_obs term is folded
                    #  into the gate biases on the host), then evict ht.
                    ph4 = psh[:, :, :].rearrange("p m (b n) -> p m b n", n=NA)
                    St = ap.tile([128, 2, NB], F32, tag="S", bufs=2)
                    ct_c = ap.tile([128, 2, NB, NA], BF16, tag="c", bufs=2,
                                   name="ct_c")
                    Sts[cur], cts[cur] = St, ct_c
                    if cur <= 0:
                        # ramp tiles: ht evict on the (still idle) scalar
                        # engine so the r/z h-part matmuls start while the
                        # DVE does the comm; per-chunk singles let each ct
                        # chunk release its ih matmul via subtile deps.
                        with tc.high_priority():
                            for m in range(2):
                                nc.scalar.activation(ht_c[:, m, :],
                                                     psh[:, m, :],
                                                     AF.Identity,
                                                     bias=bobs[:, m:m + 1])
                            for k in range(2):
                                nc.vector.reduce_sum(
                                    out=St[:, k, :], in_=ph4[:, k, :, :],
                                    axis=mybir.AxisListType.X)
                                nc.vector.tensor_sub(
                                    ct_c[:, k, :, :],
                                    St[:, k, :, None].broadcast_to(
                                        [128, NB, NA]),
                                    ph4[:, k, :, :])
                    else:
                        # steady tiles: one instruction per op class; the
                        # DVE pays ~150 fixed cycles per op, pairs save
                        # ~0.4us/tile
                        with tc.high_priority():
                            nc.vector.reduce_sum(out=St, in_=ph4,
                                                 axis=mybir.AxisListType.X)
                            nc.vector.tensor_sub(
                                ct_c,
                                St[:, :, :, None].broadcast_to(
                                    [128, 2, NB, NA]),
                                ph4)
                        nc.vector.tensor_add(
                            ht_c, psh,
                            bobs[:, 0:2, None].broadcast_to([128, 2, TT]))

                # -- tensor: encoder for the next tile (psx shares the psH
                # rotation, freed by the early-step relu, so this can be
                # scheduled mid-step instead of at the tail)
                if 0 <= nxt < NT:
                    emit_enc(nxt)
                if s == 0:
                    # extra clock-warming filler for the tile-0 fill gaps
                    warm_fill(3)

                # -- tensor: gh (n-gate h-part); gpsimd writes tt in place
                if 0 <= g < NT:
                    ht, ct = hts[g], cts[g]
                    pgh = [None] * 2
                    for m in range(2):
                        p = psG.tile([128, TT], F32, tag="g")
                        pgh[m] = p
                        for k in range(2):
                            nc.tensor.matmul(p, whhn[:, k, ts(m, 128)],
                                             ht[:, k, :], start=(k == 0),
                                             stop=(k == 1))
                    ps_ghs[g] = pgh
                    # vector: tt = (gh + b_hn) * r written IN PLACE onto the
                    # gh PSUM bank; the gi matmuls then accumulate on top and
                    # tanh reads the final value with the b_in bias.
                    for m in range(2):
                        nc.vector.scalar_tensor_tensor(
                            out=pgh[m], in0=pgh[m],
                            scalar=bhn[:, m:m + 1], in1=rts[g][:, m, :],
                            op0=ADD, op1=MULT)

                    # tensor: z gates
                    zt = ap.tile([128, 2, TT], BF16, tag="zt")
                    zts[g] = zt
                    for j in range(2):
                        p = psG.tile([128, TT], F32, tag="g")
                        for k in range(2):
                            nc.tensor.matmul(p, whhrz[:, k, ts(2 + j, 128)],
                                             ht[:, k, :], start=(k == 0),
                                             stop=False)
                        for k in range(2):
                            nc.tensor.matmul(p, wihrz[:, k, ts(2 + j, 128)],
                                             ct[:, k, :, :], start=False,
                                             stop=(k == 1))
                        nc.scalar.activation(zt[:, j, :], p, AF.Sigmoid,
                                             bias=brz[:, 2 + j:3 + j])

                    # tensor: gi accumulates onto tt in the gh banks
                    nt_ = ap.tile([128, 2, TT], BF16, tag="nt", bufs=2)
                    nts[g] = nt_
                    for m in range(2):
                        for k in range(2):
                            nc.tensor.matmul(pgh[m], wihn[:, k, ts(m, 128)],
                                             ct[:, k, :, :], start=False,
                                             stop=(k == 1))
                        # scalar: n = tanh(tt + gi + b_in)
                        nc.scalar.activation(nt_[:, m, :], pgh[m], AF.Tanh,
                                             bias=bin_[:, m:m + 1])

                    # GRU blend: d = h - n, e = z*d. The final h' = n + e is
                    # absorbed into the value head (out = Wvd@n + Wvd@e) so
                    # no h2 materialization is needed. Last tile's m=1 chunk
                    # runs on vector to halve drain latency.
                    dt_ = ap.tile([128, 2, TT], BF16, tag="dt")
                    et = ap.tile([128, 2, TT], BF16, tag="et", bufs=2)
                    h2s[g] = et
                    for m in range(2):
                        eng = (nc.vector if (g == NT - 1
                                             or (g == NT - 2 and m == 1))
                               else nc.gpsimd)
                        eng.tensor_sub(dt_[:, m, :], hts[g][:, m, :],
                                       nts[g][:, m, :])
                        eng.tensor_mul(et[:, m, :], zts[g][:, m, :],
                                       dt_[:, m, :])

                # -- tensor: value head for tile v; scalar evict + DMA out
                if 0 <= v < NT:
                    # value PSUM from the psH pool: drops psG to 6 allocs
                    # per step so the z/gh matmuls stop waiting on sigmoid
                    # bank releases
                    ps_o2 = psH.tile([128, 2, TT], F32, tag="h", name="ps_o")
                    ps_o = ps_o2[:, 0, :]
                    for k in range(2):
                        nc.tensor.matmul(ps_o[:64, :], wvd[:, k, :],
                                         nts[v][:, k, :], start=(k == 0),
                                         stop=False)
                    for k in range(2):
                        nc.tensor.matmul(ps_o[:64, :], wvd[:, k, :],
                                         h2s[v][:, k, :], start=False,
                                         stop=(k == 1))
                    osb = iop.tile([64, TT], F32, tag="osb", bufs=2)
                    if v == NT - 1:
                        # last tile: evict + store in halves so the final
                        # DMA overlaps the eviction instead of serializing
                        half = TT // 2
                        for hh in range(2):
                            sl = slice(hh * half, (hh + 1) * half)
                            if hh == 0:
                                nc.scalar.activation(osb[:, sl],
                                                     ps_o[:64, sl],
                                                     AF.Identity,
                                                     bias=bvd[:, 0:1])
                                eng = nc.sync
                            else:
                                nc.vector.tensor_scalar_add(osb[:, sl],
                                                            ps_o[:64, sl],
                                                            bvd[:, 0:1])
                                eng = nc.scalar
                            eng.dma_start(
                                out=out_d[:, v * TT + hh * half:
                                          v * TT + (hh + 1) * half],
                                in_=osb[:, sl])
                    elif v == NT - 2:
                        # this evict would sit between stt(3) and the last
                        # blend in the DVE queue and delay the drain; the
                        # scalar engine is free here
                        nc.scalar.activation(osb, ps_o[:64, :], AF.Identity,
                                             bias=bvd[:, 0:1])
                        nc.sync.dma_start(out=out_d[:, ts(v, TT)], in_=osb)
                    else:
                        nc.vector.tensor_scalar_add(osb, ps_o[:64, :],
                                                    bvd[:, 0:1])
                        nc.sync.dma_start(out=out_d[:, ts(v, TT)], in_=osb)

            # encoder for tile 0 + a couple of fill matmuls to cover the
            # relu roundtrip before the step loop starts obs(0)
            emit_enc(0)
            warm_fill(2)
            for s in range(NT + 2):
                emit_step(s)

    nc.finalize()
    return nc


def _prep_maps(obs, W_enc, b_enc, W_obs, b_obs, W_ih, b_ih, W_hh, b_hh,
               W_val, b_val, W_dec, b_dec):
    f = np.float32
    obs = np.asarray(obs, f)
    tok = np.ascontiguousarray(obs.reshape(B * NA, D_IN))

    def rr(a):
        # Matmul operands are bf16 on device; round once on host (RNE).
        return np.ascontiguousarray(np.asarray(a, f).astype(ml_dtypes.bfloat16))

    def kc(w):  # [K, M] -> [128, K/128, M] partition-major k-chunks
        k, m = w.shape
        return np.ascontiguousarray(w.reshape(k // 128, 128, m).transpose(1, 0, 2))

    W_ih = np.asarray(W_ih, np.float64)
    W_hh = np.asarray(W_hh, np.float64)
    scale = 1.0 / NA

    wenc = rr(np.ascontiguousarray(np.asarray(W_enc, f).T).reshape(128, 2, 128))
    wobs = rr(kc(np.asarray(W_obs, f).T.astype(f)))
    wihrz = rr(kc((W_ih[:512].T * scale).astype(f)))
    whhrz = rr(kc(W_hh[:512].T.astype(f)))
    wihn = rr(kc((W_ih[512:].T * scale).astype(f)))
    whhn = rr(kc(W_hh[512:].T.astype(f)))
    W_vd = (np.asarray(W_dec, np.float64) @ np.asarray(W_val, np.float64))
    wvd = rr(kc(W_vd.T.astype(f)))
    b_vd = (np.asarray(W_dec, np.float64) @ np.asarray(b_val, np.float64)
            + np.asarray(b_dec, np.float64)).astype(f)

    def bc(b, n):  # [n*128] -> [128, n]
        return np.ascontiguousarray(np.asarray(b, f).reshape(n, 128).T)

    # The on-device comm term is c' = sum_a h0 - h0 computed on the obs-layer
    # PSUM (pre-bias h0); the missing 63/64 * b_obs contribution is folded
    # through W_ih into the gate biases here.
    cobs = np.asarray(b_obs, np.float64) * (63.0 / 64.0)
    gi_fold = W_ih @ cobs          # [768]
    brz_f = (np.asarray(b_ih, np.float64)
             + np.asarray(b_hh, np.float64))[:512] + gi_fold[:512]
    bin_f = np.asarray(b_ih, np.float64)[512:] + gi_fold[512:]
    bias_pack = np.concatenate([
        bc(b_enc, 2), bc(b_obs, 2),
        bc(brz_f.astype(f), 4),
        bc(bin_f.astype(f), 2),
        bc(np.asarray(b_hh, f)[512:], 2),
    ], axis=1)  # [128, 12]
    shared = {
        "wenc": wenc, "wobs": wobs, "wihrz": wihrz, "whhrz": whhrz,
        "wihn": wihn, "whhn": whhn, "wvd": wvd,
        "bias": np.ascontiguousarray(bias_pack),
        "bvd": np.ascontiguousarray(b_vd.reshape(64, 1)),
    }
    in_maps = []
    for ci in range(N_CORES):
        sh = rr(np.ascontiguousarray(tok[ci * T_C:(ci + 1) * T_C].T))  # [128, T_C]
        in_maps.append({**shared, "obs_t": sh})
    return in_maps


def kernel(**inputs):
    global LAST_EXEC_NS, LAST_RESULTS
    if "nc" not in _PROGRAM_CACHE:
        _PROGRAM_CACHE["nc"] = _build_program()
    nc = _PROGRAM_CACHE["nc"]

    in_maps = _prep_maps(**inputs)
    res = run_bass_kernel_spmd(nc, in_maps, list(range(N_CORES)), trace=TRACE)
    LAST_EXEC_NS = res.exec_time_ns
    LAST_RESULTS = res

    parts = []
    for ci in range(N_CORES):
        o = res.results[ci]["out_t"]            # [64, T_C]
        parts.append(np.ascontiguousarray(o.T).reshape(B // N_CORES, NA, H2))
    return np.concatenate(parts, axis=0).astype(np.float32)



# revision 35
# speedup vs baseline: 1.0179x; 1.0127x over previous
"""CommNet forward pass on 8 Trainium2 NeuronCores.

Data-parallel over the batch dim: 256 batch elems -> 32 per core
(= 2048 tokens of 64 agents each). All weights replicated per core.

Device layout is feature-major: activations live in SBUF as
[feature_chunk(128 partitions), tokens]. Host pre-transposes obs and all
weights so every DMA is contiguous, and folds:
  - the 1/N comm scaling into W_ih,
  - b_ih + b_hh for the r/z gates,
  - the (linear) value+decoder layers into one W_vd = W_dec @ W_val.

The kernel is software-pipelined over 4 token tiles of 512: in step s the
tensor engine runs gates for tile s-1, encoder+obs for tile s and the value
head for tile s-2, so it never waits on the vector/scalar chain of a single
tile. Key device tricks:
  - gate chunks accumulate into 4 rotating single-bank PSUM tiles, evicted
    immediately; the obs/enc PSUM share one double-buffered rotation;
  - the comm reduce/sub runs directly on the obs-layer PSUM (pre-bias), with
    the missing 63/64*b_obs term folded into the gate biases on the host;
  - the n-gate needs no second PSUM bank: tt = (gh+b_hn)*r is written in
    place onto the gh bank by the DVE and the gi matmuls accumulate on top,
    so tanh reads the finished pre-activation straight from PSUM;
  - the GRU blend's final add is absorbed into the value head
    (out = W_vd@n + W_vd@(z*(h-n))), and the z*(h-n) product is split
    between the gpsimd and vector engines;
  - dummy matmuls during the initial weight-DMA window pre-warm the PE HAM
    clock gate, and dummy activations hoist the act-table loads into the
    same dead time.
"""

import numpy as np
import ml_dtypes

import concourse.bass as bass
import concourse.bacc as bacc
import concourse.mybir as mybir
import concourse.tile as tile
from concourse.bass import ts
from concourse.bass_utils import run_bass_kernel_spmd

N_CORES = 8
B, NA, D_IN = 256, 64, 128     # batch, agents, input dim
H0 = 256                       # hidden dim
H2 = 64                        # output dim
T_C = B * NA // N_CORES        # tokens per core (2048)
TT = 512                       # token tile (= max fp32 PSUM bank width)
NT = T_C // TT                 # token tiles per core (4)
NB = TT // NA                  # batch elems per token tile (8)
N_WARM = 5                     # HAM clock-gate warmup matmuls

F32 = mybir.dt.float32
BF16 = mybir.dt.bfloat16
AF = mybir.ActivationFunctionType
ADD = mybir.AluOpType.add
MULT = mybir.AluOpType.mult

# Set by test harness to collect a profile; kernel() stores timing here.
TRACE = False
LAST_EXEC_NS = None
LAST_RESULTS = None

_PROGRAM_CACHE = {}


def _build_program():
    nc = bacc.Bacc("TRN2", target_bir_lowering=False)

    obs_d = nc.dram_tensor("obs_t", [128, T_C], BF16, kind="ExternalInput")
    wenc_d = nc.dram_tensor("wenc", [128, 2, 128], BF16, kind="ExternalInput")
    wobs_d = nc.dram_tensor("wobs", [128, 2, 256], BF16, kind="ExternalInput")
    wihrz_d = nc.dram_tensor("wihrz", [128, 2, 512], BF16, kind="ExternalInput")
    whhrz_d = nc.dram_tensor("whhrz", [128, 2, 512], BF16, kind="ExternalInput")
    wihn_d = nc.dram_tensor("wihn", [128, 2, 256], BF16, kind="ExternalInput")
    whhn_d = nc.dram_tensor("whhn", [128, 2, 256], BF16, kind="ExternalInput")
    wvd_d = nc.dram_tensor("wvd", [128, 2, 64], BF16, kind="ExternalInput")
    bias_d = nc.dram_tensor("bias", [128, 12], F32, kind="ExternalInput")
    bvd_d = nc.dram_tensor("bvd", [64, 1], F32, kind="ExternalInput")
    out_d = nc.dram_tensor("out_t", [64, T_C], F32, kind="ExternalOutput")

    with tile.TileContext(nc) as tc:
        with (
            tc.tile_pool(name="wpool", bufs=1) as wp,
            tc.tile_pool(name="io", bufs=1) as iop,
            tc.tile_pool(name="act", bufs=1) as ap,
            tc.tile_pool(name="psG", bufs=4, space="PSUM") as psG,
            tc.tile_pool(name="psH", bufs=2, space="PSUM") as psH,
        ):
            # ---- DMA prologue: split weights over 3 queues ----
            def wload(dram, shape, tag, dt=BF16, eng=nc.gpsimd):
                t = wp.tile(shape, dt, tag=tag)
                eng.dma_start(out=t, in_=dram[:])
                return t

            # The scalar queue is kept DMA-free: its act-table loads would
            # otherwise delay weight issue.
            # obs tile 0 is the fill critical path: land it as two parallel
            # half-DMAs on different queues so the encoder can start on the
            # first half ~0.7us sooner.
            obs_sb = []
            o0 = iop.tile([128, TT], BF16, tag="obs0")
            nc.sync.dma_start(out=o0[:, 0:TT // 2], in_=obs_d[:, 0:TT // 2])
            nc.scalar.dma_start(out=o0[:, TT // 2:TT],
                                in_=obs_d[:, TT // 2:TT])
            obs_sb.append(o0)
            wenc = wload(wenc_d, [128, 2, 128], "wenc", eng=nc.gpsimd)
            whhrz = wload(whhrz_d, [128, 2, 512], "whhrz", eng=nc.sync)
            bias = wload(bias_d, [128, 12], "bias", F32, eng=nc.sync)
            wihrz = wload(wihrz_d, [128, 2, 512], "wihrz", eng=nc.gpsimd)
            wobs = wload(wobs_d, [128, 2, 256], "wobs", eng=nc.sync)
            whhn = wload(whhn_d, [128, 2, 256], "whhn", eng=nc.sync)
            wihn = wload(wihn_d, [128, 2, 256], "wihn", eng=nc.gpsimd)
            wvd = wload(wvd_d, [128, 2, 64], "wvd", eng=nc.gpsimd)
            bvd = wload(bvd_d, [64, 1], "bvd", F32, eng=nc.gpsimd)
            for i in range(1, NT):
                o = iop.tile([128, TT], BF16, tag=f"obs{i}")
                nc.sync.dma_start(out=o, in_=obs_d[:, ts(i, TT)])
                obs_sb.append(o)

            benc, bobs, brz = bias[:, 0:2], bias[:, 2:4], bias[:, 4:8]
            bin_, bhn = bias[:, 8:10], bias[:, 10:12]

            # ---- act-table hoist + PE clock warmup (runs in DMA dead time)
            warm = ap.tile([128, TT], BF16, tag="warm")
            nc.vector.memset(warm, 0.0)
            dump = ap.tile([128, 4], F32, tag="dump")
            nc.scalar.activation(dump[:, 0:1], warm[:, 0:1], AF.Sigmoid)
            nc.scalar.activation(dump[:, 1:2], warm[:, 0:1], AF.Tanh)
            nc.scalar.activation(dump[:, 2:3], warm[:, 0:1], AF.Relu)
            nc.scalar.activation(dump[:, 3:4], warm[:, 0:1], AF.Identity)
            ps_warm = psG.tile([128, TT], F32, tag="g")
            for _ in range(N_WARM):
                nc.tensor.matmul(ps_warm, warm[:, 0:128], warm,
                                 start=True, stop=True)

            # ---- per-tile state ----
            xts = [None] * NT
            hts = [None] * NT
            cts = [None] * NT
            Sts = [None] * NT
            rts = [None] * NT
            zts = [None] * NT
            tts = [None] * NT
            t2s = [None] * NT
            nts = [None] * NT
            h2s = [None] * NT
            ps_ghs = [None] * NT
            ps_gis = [None] * NT
            ps_rz = [None] * NT

            def warm_fill(n):
                for _ in range(n):
                    nc.tensor.matmul(ps_warm, warm[:, 0:128], warm,
                                     start=True, stop=True)

            def emit_enc(t):
                psx = psH.tile([128, 2, TT], F32, tag="h", name="psx")
                if t == 0:
                    # per-half matmuls, m-major: relu(m0) can start two
                    # matmuls earlier
                    half = TT // 2
                    for m in range(2):
                        for hh in range(2):
                            sl = slice(hh * half, (hh + 1) * half)
                            nc.tensor.matmul(psx[:, m, sl], wenc[:, m, :],
                                             obs_sb[t][:, sl],
                                             start=True, stop=True)
                else:
                    for m in range(2):
                        nc.tensor.matmul(psx[:, m, :], wenc[:, m, :],
                                         obs_sb[t], start=True, stop=True)
                xt = ap.tile([128, 2, TT], BF16, tag="x", bufs=2, name="xt")
                xts[t] = (psx, xt)

            def emit_step(s):
                g = s - 1       # gate tile
                cur = s         # obs/comm tile (enc ran at end of step s-1)
                v = s - 2       # value-head tile
                nxt = s + 1     # encoder tile emitted at the tail

                # -- scalar head: relu for cur (enc PSUM from prev step).
                # high_priority: the greedy scheduler otherwise parks relu
                # behind the gate sigmoids and the obs matmuls stall on it.
                if 0 <= cur < NT:
                    psx, xt = xts[cur]
                    with tc.high_priority():
                        for m in range(2):
                            nc.scalar.activation(xt[:, m, :], psx[:, m, :],
                                                 AF.Relu,
                                                 bias=benc[:, m:m + 1])

                # -- tensor: r gates (h-parts then c-parts, chunk-serial)
                if 0 <= g < NT:
                    ht, ct = hts[g], cts[g]
                    rt = ap.tile([128, 2, TT], BF16, tag="rt")
                    rts[g] = rt
                    for j in range(2):          # r0, r1
                        p = psG.tile([128, TT], F32, tag="g")
                        for k in range(2):
                            nc.tensor.matmul(p, whhrz[:, k, ts(j, 128)],
                                             ht[:, k, :], start=(k == 0),
                                             stop=False)
                        for k in range(2):
                            nc.tensor.matmul(p, wihrz[:, k, ts(j, 128)],
                                             ct[:, k, :, :], start=False,
                                             stop=(k == 1))
                        # sigmoid right after each chunk: frees the PSUM
                        # bank one matmul group earlier
                        nc.scalar.activation(rt[:, j, :], p,
                                             AF.Sigmoid, bias=brz[:, j:j + 1])

                # -- tensor: obs matmuls for cur; vector: comm off PSUM
                if 0 <= cur < NT:
                    psx, xt = xts[cur]
                    psh = psH.tile([128, 2, TT], F32, tag="h", name="psh")
                    ht_c = ap.tile([128, 2, TT], BF16, tag="h2sb", bufs=2,
                                   name="ht_c")
                    hts[cur] = ht_c
                    for m in range(2):
                        for k in range(2):
                            nc.tensor.matmul(psh[:, m, :],
                                             wobs[:, k, ts(m, 128)],
                                             xt[:, k, :], start=(k == 0),
                                             stop=(k == 1))
                    if s == 0:
                        warm_fill(4)    # keep the PE busy through comm fill
                    # vector: comm reduce + broadcast-sub straight off PSUM
                    # (c' = sum_a h0 - h0; the 63/64*b_obs term is folded
                    #  into the gate biases on the host), then evict ht.
                    ph4 = psh[:, :, :].rearrange("p m (b n) -> p m b n", n=NA)
                    St = ap.tile([128, 2, NB], F32, tag="S", bufs=2)
                    ct_c = ap.tile([128, 2, NB, NA], BF16, tag="c", bufs=2,
                                   name="ct_c")
                    Sts[cur], cts[cur] = St, ct_c
                    if cur <= 1:
                        # ramp tiles: ht evict on the (still idle) scalar
                        # engine so the r/z h-part matmuls start while the
                        # DVE does the comm; per-chunk singles let each ct
                        # chunk release its ih matmul via subtile deps.
                        with tc.high_priority():
                            for m in range(2):
                                nc.scalar.activation(ht_c[:, m, :],
                                                     psh[:, m, :],
                                                     AF.Identity,
                                                     bias=bobs[:, m:m + 1])
                            for k in range(2):
                                nc.vector.reduce_sum(
                                    out=St[:, k, :], in_=ph4[:, k, :, :],
                                    axis=mybir.AxisListType.X)
                                nc.vector.tensor_sub(
                                    ct_c[:, k, :, :],
                                    St[:, k, :, None].broadcast_to(
                                        [128, NB, NA]),
                                    ph4[:, k, :, :])
                    else:
                        # steady tiles: one instruction per op class; the
                        # DVE pays ~150 fixed cycles per op, pairs save
                        # ~0.4us/tile
                        with tc.high_priority():
                            nc.vector.reduce_sum(out=St, in_=ph4,
                                                 axis=mybir.AxisListType.X)
                            nc.vector.tensor_sub(
                                ct_c,
                                St[:, :, :, None].broadcast_to(
                                    [128, 2, NB, NA]),
                                ph4)
                        nc.vector.tensor_add(
                            ht_c, psh,
                            bobs[:, 0:2, None].broadcast_to([128, 2, TT]))

                # -- tensor: encoder for the next tile (psx shares the psH
                # rotation, freed by the early-step relu, so this can be
                # scheduled mid-step instead of at the tail)
                if 0 <= nxt < NT:
                    emit_enc(nxt)
                if s == 0:
                    # extra clock-warming filler for the tile-0 fill gaps
                    warm_fill(3)

                # -- tensor: gh (n-gate h-part); gpsimd writes tt in place
                if 0 <= g < NT:
                    ht, ct = hts[g], cts[g]
                    pgh = [None] * 2
                    for m in range(2):
                        p = psG.tile([128, TT], F32, tag="g")
                        pgh[m] = p
                        for k in range(2):
                            nc.tensor.matmul(p, whhn[:, k, ts(m, 128)],
                                             ht[:, k, :], start=(k == 0),
                                             stop=(k == 1))
                    ps_ghs[g] = pgh
                    # vector: tt = (gh + b_hn) * r written IN PLACE onto the
                    # gh PSUM bank; the gi matmuls then accumulate on top and
                    # tanh reads the final value with the b_in bias.
                    for m in range(2):
                        nc.vector.scalar_tensor_tensor(
                            out=pgh[m], in0=pgh[m],
                            scalar=bhn[:, m:m + 1], in1=rts[g][:, m, :],
                            op0=ADD, op1=MULT)

                    # tensor: z gates
                    zt = ap.tile([128, 2, TT], BF16, tag="zt")
                    zts[g] = zt
                    for j in range(2):
                        p = psG.tile([128, TT], F32, tag="g")
                        for k in range(2):
                            nc.tensor.matmul(p, whhrz[:, k, ts(2 + j, 128)],
                                             ht[:, k, :], start=(k == 0),
                                             stop=False)
                        for k in range(2):
                            nc.tensor.matmul(p, wihrz[:, k, ts(2 + j, 128)],
                                             ct[:, k, :, :], start=False,
                                             stop=(k == 1))
                        nc.scalar.activation(zt[:, j, :], p, AF.Sigmoid,
                                             bias=brz[:, 2 + j:3 + j])

                    # tensor: gi accumulates onto tt in the gh banks
                    nt_ = ap.tile([128, 2, TT], BF16, tag="nt", bufs=2)
                    nts[g] = nt_
                    for m in range(2):
                        for k in range(2):
                            nc.tensor.matmul(pgh[m], wihn[:, k, ts(m, 128)],
                                             ct[:, k, :, :], start=False,
                                             stop=(k == 1))
                        # scalar: n = tanh(tt + gi + b_in)
                        nc.scalar.activation(nt_[:, m, :], pgh[m], AF.Tanh,
                                             bias=bin_[:, m:m + 1])

                    # GRU blend: d = h - n, e = z*d. The final h' = n + e is
                    # absorbed into the value head (out = Wvd@n + Wvd@e) so
                    # no h2 materialization is needed. Last tile's m=1 chunk
                    # runs on vector to halve drain latency.
                    dt_ = ap.tile([128, 2, TT], BF16, tag="dt")
                    et = ap.tile([128, 2, TT], BF16, tag="et", bufs=2)
                    h2s[g] = et
                    for m in range(2):
                        eng = (nc.vector if (g == NT - 1
                                             or (g == NT - 2 and m == 1))
                               else nc.gpsimd)
                        eng.tensor_sub(dt_[:, m, :], hts[g][:, m, :],
                                       nts[g][:, m, :])
                        eng.tensor_mul(et[:, m, :], zts[g][:, m, :],
                                       dt_[:, m, :])

                # -- tensor: value head for tile v; scalar evict + DMA out
                if 0 <= v < NT:
                    # value PSUM from the psH pool: drops psG to 6 allocs
                    # per step so the z/gh matmuls stop waiting on sigmoid
                    # bank releases
                    ps_o2 = psH.tile([128, 2, TT], F32, tag="h", name="ps_o")
                    ps_o = ps_o2[:, 0, :]
                    for k in range(2):
                        nc.tensor.matmul(ps_o[:64, :], wvd[:, k, :],
                                         nts[v][:, k, :], start=(k == 0),
                                         stop=False)
                    for k in range(2):
                        nc.tensor.matmul(ps_o[:64, :], wvd[:, k, :],
                                         h2s[v][:, k, :], start=False,
                                         stop=(k == 1))
                    osb = iop.tile([64, TT], F32, tag="osb", bufs=2)
                    if v == NT - 1:
                        # last tile: evict + store in halves so the final
                        # DMA overlaps the eviction instead of serializing
                        half = TT // 2
                        for hh in range(2):
                            sl = slice(hh * half, (hh + 1) * half)
                            if hh == 0:
                                nc.scalar.activation(osb[:, sl],
                                                     ps_o[:64, sl],
                                                     AF.Identity,
                                                     bias=bvd[:, 0:1])
                                eng = nc.sync
                            else:
                                nc.vector.tensor_scalar_add(osb[:, sl],
                                                            ps_o[:64, sl],
                                                            bvd[:, 0:1])
                                eng = nc.scalar
                            eng.dma_start(
                                out=out_d[:, v * TT + hh * half:
                                          v * TT + (hh + 1) * half],
                                in_=osb[:, sl])
                    elif v == NT - 2:
                        # this evict would sit between stt(3) and the last
                        # blend in the DVE queue and delay the drain; the
                        # scalar engine is free here
                        nc.scalar.activation(osb, ps_o[:64, :], AF.Identity,
                                             bias=bvd[:, 0:1])
                        nc.sync.dma_start(out=out_d[:, ts(v, TT)], in_=osb)
                    else:
                        nc.vector.tensor_scalar_add(osb, ps_o[:64, :],
                                                    bvd[:, 0:1])
                        nc.sync.dma_start(out=out_d[:, ts(v, TT)], in_=osb)

            # encoder for tile 0 + a couple of fill matmuls to cover the
            # relu roundtrip before the step loop starts obs(0)
            emit_enc(0)
            warm_fill(2)
            for s in range(NT + 2):
                emit_step(s)

    nc.finalize()
    return nc


def _prep_maps(obs, W_enc, b_enc, W_obs, b_obs, W_ih, b_ih, W_hh, b_hh,
               W_val, b_val, W_dec, b_dec):
    f = np.float32
    obs = np.asarray(obs, f)
    tok = np.ascontiguousarray(obs.reshape(B * NA, D_IN))

    def rr(a):
        # Matmul operands are bf16 on device; round once on host (RNE).
        return np.ascontiguousarray(np.asarray(a, f).astype(ml_dtypes.bfloat16))

    def kc(w):  # [K, M] -> [128, K/128, M] partition-major k-chunks
        k, m = w.shape
        return np.ascontiguousarray(w.reshape(k // 128, 128, m).transpose(1, 0, 2))

    W_ih = np.asarray(W_ih, np.float64)
    W_hh = np.asarray(W_hh, np.float64)
    scale = 1.0 / NA

    wenc = rr(np.ascontiguousarray(np.asarray(W_enc, f).T).reshape(128, 2, 128))
    wobs = rr(kc(np.asarray(W_obs, f).T.astype(f)))
    wihrz = rr(kc((W_ih[:512].T * scale).astype(f)))
    whhrz = rr(kc(W_hh[:512].T.astype(f)))
    wihn = rr(kc((W_ih[512:].T * scale).astype(f)))
    whhn = rr(kc(W_hh[512:].T.astype(f)))
    W_vd = (np.asarray(W_dec, np.float64) @ np.asarray(W_val, np.float64))
    wvd = rr(kc(W_vd.T.astype(f)))
    b_vd = (np.asarray(W_dec, np.float64) @ np.asarray(b_val, np.float64)
            + np.asarray(b_dec, np.float64)).astype(f)

    def bc(b, n):  # [n*128] -> [128, n]
        return np.ascontiguousarray(np.asarray(b, f).reshape(n, 128).T)

    # The on-device comm term is c' = sum_a h0 - h0 computed on the obs-layer
    # PSUM (pre-bias h0); the missing 63/64 * b_obs contribution is folded
    # through W_ih into the gate biases here.
    cobs = np.asarray(b_obs, np.float64) * (63.0 / 64.0)
    gi_fold = W_ih @ cobs          # [768]
    brz_f = (np.asarray(b_ih, np.float64)
             + np.asarray(b_hh, np.float64))[:512] + gi_fold[:512]
    bin_f = np.asarray(b_ih, np.float64)[512:] + gi_fold[512:]
    bias_pack = np.concatenate([
        bc(b_enc, 2), bc(b_obs, 2),
        bc(brz_f.astype(f), 4),
        bc(bin_f.astype(f), 2),
        bc(np.asarray(b_hh, f)[512:], 2),
    ], axis=1)  # [128, 12]
    shared = {
        "wenc": wenc, "wobs": wobs, "wihrz": wihrz, "whhrz": whhrz,
        "wihn": wihn, "whhn": whhn, "wvd": wvd,
        "bias": np.ascontiguousarray(bias_pack),
        "bvd": np.ascontiguousarray(b_vd.reshape(64, 1)),
    }
    in_maps = []
    for ci in range(N_CORES):
        sh = rr(np.ascontiguousarray(tok[ci * T_C:(ci + 1) * T_C].T))  # [128, T_C]
        in_maps.append({**shared, "obs_t": sh})
    return in_maps


def kernel(**inputs):
    global LAST_EXEC_NS, LAST_RESULTS
    if "nc" not in _PROGRAM_CACHE:
        _PROGRAM_CACHE["nc"] = _build_program()
    nc = _PROGRAM_CACHE["nc"]

    in_maps = _prep_maps(**inputs)
    res = run_bass_kernel_spmd(nc, in_maps, list(range(N_CORES)), trace=TRACE)
    LAST_EXEC_NS = res.exec_time_ns
    LAST_RESULTS = res

    parts = []
    for ci in range(N_CORES):
        o = res.results[ci]["out_t"]            # [64, T_C]
        parts.append(np.ascontiguousarray(o.T).reshape(B // N_CORES, NA, H2))
    return np.concatenate(parts, axis=0).astype(np.float32)



# revision 37
# speedup vs baseline: 1.0180x; 1.0001x over previous
"""CommNet forward pass on 8 Trainium2 NeuronCores.

Data-parallel over the batch dim: 256 batch elems -> 32 per core
(= 2048 tokens of 64 agents each). All weights replicated per core.

Device layout is feature-major: activations live in SBUF as
[feature_chunk(128 partitions), tokens]. Host pre-transposes obs and all
weights so every DMA is contiguous, and folds:
  - the 1/N comm scaling into W_ih,
  - b_ih + b_hh for the r/z gates,
  - the (linear) value+decoder layers into one W_vd = W_dec @ W_val.

The kernel is software-pipelined over 4 token tiles of 512: in step s the
tensor engine runs gates for tile s-1, encoder+obs for tile s and the value
head for tile s-2, so it never waits on the vector/scalar chain of a single
tile. Key device tricks:
  - gate chunks accumulate into 4 rotating single-bank PSUM tiles, evicted
    immediately; the obs/enc PSUM and the value-head PSUM share one
    double-buffered pair rotation (keeping the gate pool at 6 allocs/step
    so the z/gh matmuls don't wait on sigmoid bank releases);
  - the comm reduce/sub runs directly on the obs-layer PSUM (pre-bias), with
    the missing 63/64*b_obs term folded into the gate biases on the host;
    steady tiles do reduce/sub/bias-evict as one paired instruction each
    (the DVE pays ~150 fixed cycles per op), while the two ramp tiles use
    per-chunk singles with the bias evict on the then-idle scalar engine so
    the first gate matmuls start ~1.5us earlier;
  - the n-gate needs no second PSUM bank: tt = (gh+b_hn)*r is written in
    place onto the gh bank by the DVE and the gi matmuls accumulate on top,
    so tanh reads the finished pre-activation straight from PSUM;
  - the GRU blend's final add is absorbed into the value head
    (out = W_vd@n + W_vd@(z*(h-n))), and the z*(h-n) product is split
    between the gpsimd and vector engines; the last two tiles keep the
    drain on the DVE/scalar (gpsimd is ~3x slower per op);
  - the output evicts for the last two tiles ride the scalar engine
    (Identity activation with the b_vd bias AP) so the DVE queue stays
    clear of the stt -> blend drain path, and the final tile stores in
    halves (scalar-ACT + sync-DMA, then DVE + scalar-DMA);
  - dummy matmuls during the initial weight-DMA window pre-warm the PE HAM
    clock gate, and dummy activations hoist the act-table loads into the
    same dead time.
"""

import numpy as np
import ml_dtypes

import concourse.bass as bass
import concourse.bacc as bacc
import concourse.mybir as mybir
import concourse.tile as tile
from concourse.bass import ts
from concourse.bass_utils import run_bass_kernel_spmd

N_CORES = 8
B, NA, D_IN = 256, 64, 128     # batch, agents, input dim
H0 = 256                       # hidden dim
H2 = 64                        # output dim
T_C = B * NA // N_CORES        # tokens per core (2048)
TT = 512                       # token tile (= max fp32 PSUM bank width)
NT = T_C // TT                 # token tiles per core (4)
NB = TT // NA                  # batch elems per token tile (8)
N_WARM = 5                     # HAM clock-gate warmup matmuls

F32 = mybir.dt.float32
BF16 = mybir.dt.bfloat16
AF = mybir.ActivationFunctionType
ADD = mybir.AluOpType.add
MULT = mybir.AluOpType.mult

# Set by test harness to collect a profile; kernel() stores timing here.
TRACE = False
LAST_EXEC_NS = None
LAST_RESULTS = None

_PROGRAM_CACHE = {}


def _build_program():
    nc = bacc.Bacc("TRN2", target_bir_lowering=False)

    obs_d = nc.dram_tensor("obs_t", [128, T_C], BF16, kind="ExternalInput")
    wenc_d = nc.dram_tensor("wenc", [128, 2, 128], BF16, kind="ExternalInput")
    wobs_d = nc.dram_tensor("wobs", [128, 2, 256], BF16, kind="ExternalInput")
    wihrz_d = nc.dram_tensor("wihrz", [128, 2, 512], BF16, kind="ExternalInput")
    whhrz_d = nc.dram_tensor("whhrz", [128, 2, 512], BF16, kind="ExternalInput")
    wihn_d = nc.dram_tensor("wihn", [128, 2, 256], BF16, kind="ExternalInput")
    whhn_d = nc.dram_tensor("whhn", [128, 2, 256], BF16, kind="ExternalInput")
    wvd_d = nc.dram_tensor("wvd", [128, 2, 64], BF16, kind="ExternalInput")
    bias_d = nc.dram_tensor("bias", [128, 12], F32, kind="ExternalInput")
    bvd_d = nc.dram_tensor("bvd", [64, 1], F32, kind="ExternalInput")
    out_d = nc.dram_tensor("out_t", [64, T_C], F32, kind="ExternalOutput")

    with tile.TileContext(nc) as tc:
        with (
            tc.tile_pool(name="wpool", bufs=1) as wp,
            tc.tile_pool(name="io", bufs=1) as iop,
            tc.tile_pool(name="act", bufs=1) as ap,
            tc.tile_pool(name="psG", bufs=4, space="PSUM") as psG,
            tc.tile_pool(name="psH", bufs=2, space="PSUM") as psH,
        ):
            # ---- DMA prologue: split weights over 3 queues ----
            def wload(dram, shape, tag, dt=BF16, eng=nc.gpsimd):
                t = wp.tile(shape, dt, tag=tag)
                eng.dma_start(out=t, in_=dram[:])
                return t

            # The scalar queue is kept DMA-free: its act-table loads would
            # otherwise delay weight issue.
            # obs tile 0 is the fill critical path: land it as two parallel
            # half-DMAs on different queues so the encoder can start on the
            # first half ~0.7us sooner.
            obs_sb = []
            o0 = iop.tile([128, TT], BF16, tag="obs0")
            nc.sync.dma_start(out=o0[:, 0:TT // 2], in_=obs_d[:, 0:TT // 2])
            nc.scalar.dma_start(out=o0[:, TT // 2:TT],
                                in_=obs_d[:, TT // 2:TT])
            obs_sb.append(o0)
            wenc = wload(wenc_d, [128, 2, 128], "wenc", eng=nc.gpsimd)
            whhrz = wload(whhrz_d, [128, 2, 512], "whhrz", eng=nc.sync)
            bias = wload(bias_d, [128, 12], "bias", F32, eng=nc.sync)
            wihrz = wload(wihrz_d, [128, 2, 512], "wihrz", eng=nc.gpsimd)
            wobs = wload(wobs_d, [128, 2, 256], "wobs", eng=nc.sync)
            whhn = wload(whhn_d, [128, 2, 256], "whhn", eng=nc.sync)
            wihn = wload(wihn_d, [128, 2, 256], "wihn", eng=nc.gpsimd)
            wvd = wload(wvd_d, [128, 2, 64], "wvd", eng=nc.gpsimd)
            bvd = wload(bvd_d, [64, 1], "bvd", F32, eng=nc.gpsimd)
            for i in range(1, NT):
                o = iop.tile([128, TT], BF16, tag=f"obs{i}")
                nc.sync.dma_start(out=o, in_=obs_d[:, ts(i, TT)])
                obs_sb.append(o)

            benc, bobs, brz = bias[:, 0:2], bias[:, 2:4], bias[:, 4:8]
            bin_, bhn = bias[:, 8:10], bias[:, 10:12]

            # ---- act-table hoist + PE clock warmup (runs in DMA dead time)
            warm = ap.tile([128, TT], BF16, tag="warm")
            nc.vector.memset(warm, 0.0)
            dump = ap.tile([128, 4], F32, tag="dump")
            nc.scalar.activation(dump[:, 0:1], warm[:, 0:1], AF.Sigmoid)
            nc.scalar.activation(dump[:, 1:2], warm[:, 0:1], AF.Tanh)
            nc.scalar.activation(dump[:, 2:3], warm[:, 0:1], AF.Relu)
            nc.scalar.activation(dump[:, 3:4], warm[:, 0:1], AF.Identity)
            ps_warm = psG.tile([128, TT], F32, tag="g")
            for _ in range(N_WARM):
                nc.tensor.matmul(ps_warm, warm[:, 0:128], warm,
                                 start=True, stop=True)

            # ---- per-tile state ----
            xts = [None] * NT
            hts = [None] * NT
            cts = [None] * NT
            Sts = [None] * NT
            rts = [None] * NT
            zts = [None] * NT
            tts = [None] * NT
            t2s = [None] * NT
            nts = [None] * NT
            h2s = [None] * NT
            ps_ghs = [None] * NT
            ps_gis = [None] * NT
            ps_rz = [None] * NT

            def warm_fill(n):
                for _ in range(n):
                    nc.tensor.matmul(ps_warm, warm[:, 0:128], warm,
                                     start=True, stop=True)

            def emit_enc(t):
                psx = psH.tile([128, 2, TT], F32, tag="h", name="psx")
                if t == 0:
                    # per-half matmuls, m-major: relu(m0) can start two
                    # matmuls earlier
                    half = TT // 2
                    for m in range(2):
                        for hh in range(2):
                            sl = slice(hh * half, (hh + 1) * half)
                            nc.tensor.matmul(psx[:, m, sl], wenc[:, m, :],
                                             obs_sb[t][:, sl],
                                             start=True, stop=True)
                else:
                    for m in range(2):
                        nc.tensor.matmul(psx[:, m, :], wenc[:, m, :],
                                         obs_sb[t], start=True, stop=True)
                xt = ap.tile([128, 2, TT], BF16, tag="x", bufs=2, name="xt")
                xts[t] = (psx, xt)

            def emit_step(s):
                g = s - 1       # gate tile
                cur = s         # obs/comm tile (enc ran at end of step s-1)
                v = s - 2       # value-head tile
                nxt = s + 1     # encoder tile emitted at the tail

                # -- scalar head: relu for cur (enc PSUM from prev step).
                # high_priority: the greedy scheduler otherwise parks relu
                # behind the gate sigmoids and the obs matmuls stall on it.
                if 0 <= cur < NT:
                    psx, xt = xts[cur]
                    with tc.high_priority():
                        for m in range(2):
                            nc.scalar.activation(xt[:, m, :], psx[:, m, :],
                                                 AF.Relu,
                                                 bias=benc[:, m:m + 1])

                # -- tensor: r gates (h-parts then c-parts, chunk-serial)
                if 0 <= g < NT:
                    ht, ct = hts[g], cts[g]
                    prz = [None] * 2
                    for j in range(2):          # r0, r1
                        p = psG.tile([128, TT], F32, tag="g")
                        prz[j] = p
                        for k in range(2):
                            nc.tensor.matmul(p, whhrz[:, k, ts(j, 128)],
                                             ht[:, k, :], start=(k == 0),
                                             stop=False)
                        for k in range(2):
                            nc.tensor.matmul(p, wihrz[:, k, ts(j, 128)],
                                             ct[:, k, :, :], start=False,
                                             stop=(k == 1))
                    # scalar: sigmoid r
                    rt = ap.tile([128, 2, TT], BF16, tag="rt")
                    rts[g] = rt
                    for j in range(2):
                        nc.scalar.activation(rt[:, j, :], prz[j],
                                             AF.Sigmoid, bias=brz[:, j:j + 1])

                # -- tensor: obs matmuls for cur; vector: comm off PSUM
                if 0 <= cur < NT:
                    psx, xt = xts[cur]
                    psh = psH.tile([128, 2, TT], F32, tag="h", name="psh")
                    ht_c = ap.tile([128, 2, TT], BF16, tag="h2sb", bufs=2,
                                   name="ht_c")
                    hts[cur] = ht_c
                    for m in range(2):
                        for k in range(2):
                            nc.tensor.matmul(psh[:, m, :],
                                             wobs[:, k, ts(m, 128)],
                                             xt[:, k, :], start=(k == 0),
                                             stop=(k == 1))
                    if s == 0:
                        warm_fill(4)    # keep the PE busy through comm fill
                    # vector: comm reduce + broadcast-sub straight off PSUM
                    # (c' = sum_a h0 - h0; the 63/64*b_obs term is folded
                    #  into the gate biases on the host), then evict ht.
                    ph4 = psh[:, :, :].rearrange("p m (b n) -> p m b n", n=NA)
                    St = ap.tile([128, 2, NB], F32, tag="S", bufs=2)
                    ct_c = ap.tile([128, 2, NB, NA], BF16, tag="c", bufs=2,
                                   name="ct_c")
                    Sts[cur], cts[cur] = St, ct_c
                    if cur <= 1:
                        # ramp tiles: ht evict on the (still idle) scalar
                        # engine so the r/z h-part matmuls start while the
                        # DVE does the comm; per-chunk singles let each ct
                        # chunk release its ih matmul via subtile deps.
                        with tc.high_priority():
                            for m in range(2):
                                nc.scalar.activation(ht_c[:, m, :],
                                                     psh[:, m, :],
                                                     AF.Identity,
                                                     bias=bobs[:, m:m + 1])
                            for k in range(2):
                                nc.vector.reduce_sum(
                                    out=St[:, k, :], in_=ph4[:, k, :, :],
                                    axis=mybir.AxisListType.X)
                                nc.vector.tensor_sub(
                                    ct_c[:, k, :, :],
                                    St[:, k, :, None].broadcast_to(
                                        [128, NB, NA]),
                                    ph4[:, k, :, :])
                    else:
                        # steady tiles: one instruction per op class; the
                        # DVE pays ~150 fixed cycles per op, pairs save
                        # ~0.4us/tile
                        with tc.high_priority():
                            nc.vector.reduce_sum(out=St, in_=ph4,
                                                 axis=mybir.AxisListType.X)
                            nc.vector.tensor_sub(
                                ct_c,
                                St[:, :, :, None].broadcast_to(
                                    [128, 2, NB, NA]),
                                ph4)
                        nc.vector.tensor_add(
                            ht_c, psh,
                            bobs[:, 0:2, None].broadcast_to([128, 2, TT]))

                # -- tensor: encoder for the next tile (psx shares the psH
                # rotation, freed by the early-step relu, so this can be
                # scheduled mid-step instead of at the tail)
                if 0 <= nxt < NT:
                    emit_enc(nxt)
                if s == 0:
                    # extra clock-warming filler for the tile-0 fill gaps
                    warm_fill(3)

                # -- tensor: gh (n-gate h-part); gpsimd writes tt in place
                if 0 <= g < NT:
                    ht, ct = hts[g], cts[g]
                    pgh = [None] * 2
                    for m in range(2):
                        p = psG.tile([128, TT], F32, tag="g")
                        pgh[m] = p
                        for k in range(2):
                            nc.tensor.matmul(p, whhn[:, k, ts(m, 128)],
                                             ht[:, k, :], start=(k == 0),
                                             stop=(k == 1))
                    ps_ghs[g] = pgh
                    # vector: tt = (gh + b_hn) * r written IN PLACE onto the
                    # gh PSUM bank; the gi matmuls then accumulate on top and
                    # tanh reads the final value with the b_in bias.
                    for m in range(2):
                        nc.vector.scalar_tensor_tensor(
                            out=pgh[m], in0=pgh[m],
                            scalar=bhn[:, m:m + 1], in1=rts[g][:, m, :],
                            op0=ADD, op1=MULT)

                    # tensor: z gates
                    zt = ap.tile([128, 2, TT], BF16, tag="zt")
                    zts[g] = zt
                    for j in range(2):
                        p = psG.tile([128, TT], F32, tag="g")
                        for k in range(2):
                            nc.tensor.matmul(p, whhrz[:, k, ts(2 + j, 128)],
                                             ht[:, k, :], start=(k == 0),
                                             stop=False)
                        for k in range(2):
                            nc.tensor.matmul(p, wihrz[:, k, ts(2 + j, 128)],
                                             ct[:, k, :, :], start=False,
                                             stop=(k == 1))
                        nc.scalar.activation(zt[:, j, :], p, AF.Sigmoid,
                                             bias=brz[:, 2 + j:3 + j])

                    # tensor: gi accumulates onto tt in the gh banks
                    nt_ = ap.tile([128, 2, TT], BF16, tag="nt", bufs=2)
                    nts[g] = nt_
                    for m in range(2):
                        for k in range(2):
                            nc.tensor.matmul(pgh[m], wihn[:, k, ts(m, 128)],
                                             ct[:, k, :, :], start=False,
                                             stop=(k == 1))
                        # scalar: n = tanh(tt + gi + b_in)
                        nc.scalar.activation(nt_[:, m, :], pgh[m], AF.Tanh,
                                             bias=bin_[:, m:m + 1])

                    # GRU blend: d = h - n, e = z*d. The final h' = n + e is
                    # absorbed into the value head (out = Wvd@n + Wvd@e) so
                    # no h2 materialization is needed. Last tile's m=1 chunk
                    # runs on vector to halve drain latency.
                    dt_ = ap.tile([128, 2, TT], BF16, tag="dt")
                    et = ap.tile([128, 2, TT], BF16, tag="et", bufs=2)
                    h2s[g] = et
                    for m in range(2):
                        eng = (nc.vector if (g == NT - 1
                                             or (g == NT - 2 and m == 1))
                               else nc.gpsimd)
                        eng.tensor_sub(dt_[:, m, :], hts[g][:, m, :],
                                       nts[g][:, m, :])
                        eng.tensor_mul(et[:, m, :], zts[g][:, m, :],
                                       dt_[:, m, :])

                # -- tensor: value head for tile v; scalar evict + DMA out
                if 0 <= v < NT:
                    # value PSUM from the psH pool: drops psG to 6 allocs
                    # per step so the z/gh matmuls stop waiting on sigmoid
                    # bank releases
                    ps_o2 = psH.tile([128, 2, TT], F32, tag="h", name="ps_o")
                    ps_o = ps_o2[:, 0, :]
                    for k in range(2):
                        nc.tensor.matmul(ps_o[:64, :], wvd[:, k, :],
                                         nts[v][:, k, :], start=(k == 0),
                                         stop=False)
                    for k in range(2):
                        nc.tensor.matmul(ps_o[:64, :], wvd[:, k, :],
                                         h2s[v][:, k, :], start=False,
                                         stop=(k == 1))
                    osb = iop.tile([64, TT], F32, tag="osb", bufs=2)
                    if v == NT - 1:
                        # last tile: evict + store in halves so the final
                        # DMA overlaps the eviction instead of serializing
                        half = TT // 2
                        for hh in range(2):
                            sl = slice(hh * half, (hh + 1) * half)
                            if hh == 0:
                                nc.scalar.activation(osb[:, sl],
                                                     ps_o[:64, sl],
                                                     AF.Identity,
                                                     bias=bvd[:, 0:1])
                                eng = nc.sync
                            else:
                                nc.vector.tensor_scalar_add(osb[:, sl],
                                                            ps_o[:64, sl],
                                                            bvd[:, 0:1])
                                eng = nc.scalar
                            eng.dma_start(
                                out=out_d[:, v * TT + hh * half:
                                          v * TT + (hh + 1) * half],
                                in_=osb[:, sl])
                    elif v == NT - 2:
                        # this evict would sit between stt(3) and the last
                        # blend in the DVE queue and delay the drain; the
                        # scalar engine is free here
                        nc.scalar.activation(osb, ps_o[:64, :], AF.Identity,
                                             bias=bvd[:, 0:1])
                        nc.sync.dma_start(out=out_d[:, ts(v, TT)], in_=osb)
                    else:
                        nc.vector.tensor_scalar_add(osb, ps_o[:64, :],
                                                    bvd[:, 0:1])
                        nc.sync.dma_start(out=out_d[:, ts(v, TT)], in_=osb)

            # encoder for tile 0 + a couple of fill matmuls to cover the
            # relu roundtrip before the step loop starts obs(0)
            emit_enc(0)
            warm_fill(2)
            for s in range(NT + 2):
                emit_step(s)

    nc.finalize()
    return nc


def _prep_maps(obs, W_enc, b_enc, W_obs, b_obs, W_ih, b_ih, W_hh, b_hh,
               W_val, b_val, W_dec, b_dec):
    f = np.float32
    obs = np.asarray(obs, f)
    tok = np.ascontiguousarray(obs.reshape(B * NA, D_IN))

    def rr(a):
        # Matmul operands are bf16 on device; round once on host (RNE).
        return np.ascontiguousarray(np.asarray(a, f).astype(ml_dtypes.bfloat16))

    def kc(w):  # [K, M] -> [128, K/128, M] partition-major k-chunks
        k, m = w.shape
        return np.ascontiguousarray(w.reshape(k // 128, 128, m).transpose(1, 0, 2))

    W_ih = np.asarray(W_ih, np.float64)
    W_hh = np.asarray(W_hh, np.float64)
    scale = 1.0 / NA

    wenc = rr(np.ascontiguousarray(np.asarray(W_enc, f).T).reshape(128, 2, 128))
    wobs = rr(kc(np.asarray(W_obs, f).T.astype(f)))
    wihrz = rr(kc((W_ih[:512].T * scale).astype(f)))
    whhrz = rr(kc(W_hh[:512].T.astype(f)))
    wihn = rr(kc((W_ih[512:].T * scale).astype(f)))
    whhn = rr(kc(W_hh[512:].T.astype(f)))
    W_vd = (np.asarray(W_dec, np.float64) @ np.asarray(W_val, np.float64))
    wvd = rr(kc(W_vd.T.astype(f)))
    b_vd = (np.asarray(W_dec, np.float64) @ np.asarray(b_val, np.float64)
            + np.asarray(b_dec, np.float64)).astype(f)

    def bc(b, n):  # [n*128] -> [128, n]
        return np.ascontiguousarray(np.asarray(b, f).reshape(n, 128).T)

    # The on-device comm term is c' = sum_a h0 - h0 computed on the obs-layer
    # PSUM (pre-bias h0); the missing 63/64 * b_obs contribution is folded
    # through W_ih into the gate biases here.
    cobs = np.asarray(b_obs, np.float64) * (63.0 / 64.0)
    gi_fold = W_ih @ cobs          # [768]
    brz_f = (np.asarray(b_ih, np.float64)
             + np.asarray(b_hh, np.float64))[:512] + gi_fold[:512]
    bin_f = np.asarray(b_ih, np.float64)[512:] + gi_fold[512:]
    bias_pack = np.concatenate([
        bc(b_enc, 2), bc(b_obs, 2),
        bc(brz_f.astype(f), 4),
        bc(bin_f.astype(f), 2),
        bc(np.asarray(b_hh, f)[512:], 2),
    ], axis=1)  # [128, 12]
    shared = {
        "wenc": wenc, "wobs": wobs, "wihrz": wihrz, "whhrz": whhrz,
        "wihn": wihn, "whhn": whhn, "wvd": wvd,
        "bias": np.ascontiguousarray(bias_pack),
        "bvd": np.ascontiguousarray(b_vd.reshape(64, 1)),
    }
    in_maps = []
    for ci in range(N_CORES):
        sh = rr(np.ascontiguousarray(tok[ci * T_C:(ci + 1) * T_C].T))  # [128, T_C]
        in_maps.append({**shared, "obs_t": sh})
    return in_maps


def kernel(**inputs):
    global LAST_EXEC_NS, LAST_RESULTS
    if "nc" not in _PROGRAM_CACHE:
        _PROGRAM_CACHE["nc"] = _build_program()
    nc = _PROGRAM_CACHE["nc"]

    in_maps = _prep_maps(**inputs)
    res = run_bass_kernel_spmd(nc, in_maps, list(range(N_CORES)), trace=TRACE)
    LAST_EXEC_NS = res.exec_time_ns
    LAST_RESULTS = res

    parts = []
    for ci in range(N_CORES):
        o = res.results[ci]["out_t"]            # [64, T_C]
        parts.append(np.ascontiguousarray(o.T).reshape(B // N_CORES, NA, H2))
    return np.concatenate(parts, axis=0).astype(np.float32)

